# revision 5
# baseline (speedup 1.0000x reference)
"""Trainium2 Bass kernel for nn_ConcatLayer_57982058496361 (topk_masking).

Per row of 9 floats (3 groups of 3):
  mi_g   = +1/0/-1 by first-argmax of the group (0 on ties)
  calc   = |mi_1| * (mi_0 + mi_1 + mi_2)
  keep_g = sign(calc) == mi_g
  idx    = 1 - sign(calc)
  vals_g = keep_g * x_g[idx]
  win    = first-argmax(vals); out = keep_win ? x_win : 0

Key identity used: for kept groups x_g[idx] equals the group max M_g,
except when sign(calc)==0 where it is the middle element b_g.

Data-parallel over 8 NeuronCores; each core processes N/8 rows.
"""

import os
import numpy as np

N_ROWS = 8388608
N_CORES = 8
ROWS_PER_CORE = N_ROWS // N_CORES  # 1048576
P = 128
F = 512                      # rows per partition per tile
TILE_ROWS = P * F
TILES = ROWS_PER_CORE // TILE_ROWS

LAST_EXEC_NS = None
LAST_RESULTS = None
_CACHE = {}


def _build_nc():
    import concourse.bacc as bacc
    import concourse.mybir as mybir
    from concourse.tile import TileContext

    f32 = mybir.dt.float32
    Alu = mybir.AluOpType

    nc = bacc.Bacc(
        "TRN2",
        target_bir_lowering=False,
        debug=False,
        num_devices=N_CORES,
    )
    x_d = nc.dram_tensor("inputs", [ROWS_PER_CORE, 9], f32, kind="ExternalInput")
    o_d = nc.dram_tensor("out", [ROWS_PER_CORE, 3], f32, kind="ExternalOutput")
    xt = x_d.rearrange("(t p f) e -> t p f e", p=P, f=F)  # [T,128,F,9]
    ot = o_d.rearrange("(t p f) e -> t p f e", p=P, f=F)  # [T,128,F,3]

    with TileContext(nc) as tc:
        with tc.tile_pool(name="io", bufs=3) as io, tc.tile_pool(name="tmp", bufs=2) as tp:
            for t in range(TILES):
                x = io.tile([P, F, 9], f32, tag="x")
                nc.sync.dma_start(x[:], xt[t])

                a = [x[:, :, 3 * g + 0] for g in range(3)]
                b = [x[:, :, 3 * g + 1] for g in range(3)]
                c = [x[:, :, 3 * g + 2] for g in range(3)]

                M, mi = [], []
                for g in range(3):
                    u1 = tp.tile([P, F], f32, tag="u1")
                    nc.vector.tensor_tensor(u1[:], b[g], c[g], Alu.max)
                    u2 = tp.tile([P, F], f32, tag="u2")
                    nc.vector.tensor_tensor(u2[:], a[g], b[g], Alu.max)
                    Mg = tp.tile([P, F], f32, tag=f"M{g}")
                    nc.vector.tensor_tensor(Mg[:], a[g], u1[:], Alu.max)
                    A = tp.tile([P, F], f32, tag="A")
                    nc.vector.tensor_tensor(A[:], a[g], u1[:], Alu.is_gt)
                    C = tp.tile([P, F], f32, tag="C")
                    nc.vector.tensor_tensor(C[:], c[g], u2[:], Alu.is_gt)
                    mig = tp.tile([P, F], f32, tag=f"mi{g}")
                    nc.vector.tensor_tensor(mig[:], A[:], C[:], Alu.subtract)
                    M.append(Mg)
                    mi.append(mig)

                s3a = tp.tile([P, F], f32, tag="s3a")
                nc.vector.tensor_tensor(s3a[:], mi[0][:], mi[1][:], Alu.add)
                s3 = tp.tile([P, F], f32, tag="s3")
                nc.vector.tensor_tensor(s3[:], s3a[:], mi[2][:], Alu.add)

                sg = tp.tile([P, F], f32, tag="sg")
                nc.scalar.sign(sg[:], s3[:])  # ACT engine

                ab = tp.tile([P, F], f32, tag="ab")
                nc.vector.tensor_scalar(ab[:], mi[1][:], 0.0, None, Alu.not_equal)
                sc = tp.tile([P, F], f32, tag="sc")
                nc.vector.tensor_tensor(sc[:], ab[:], sg[:], Alu.mult)
                u8 = mybir.dt.uint8
                i1 = tp.tile([P, F], u8, tag="i1")
                nc.vector.tensor_scalar(i1[:], sc[:], 0.0, None, Alu.is_equal)

                keep, vals = [], []
                for g in range(3):
                    # where sign(calc)==0, the kept value is the middle element
                    nc.vector.copy_predicated(M[g][:], i1[:], b[g])
                    kg = tp.tile([P, F], f32, tag=f"k{g}")
                    nc.vector.tensor_tensor(kg[:], mi[g][:], sc[:], Alu.is_equal)
                    vg = tp.tile([P, F], f32, tag=f"v{g}")
                    nc.vector.tensor_tensor(vg[:], kg[:], M[g][:], Alu.mult)
                    keep.append(kg)
                    vals.append(vg)

                wm = tp.tile([P, F], f32, tag="wm")
                nc.vector.tensor_tensor(wm[:], vals[0][:], vals[1][:], Alu.max)
                wm2 = tp.tile([P, F], f32, tag="wm2")
                nc.vector.tensor_tensor(wm2[:], wm[:], vals[2][:], Alu.max)

                m = []
                for g in range(3):
                    eg = tp.tile([P, F], f32, tag="eg")
                    nc.vector.tensor_tensor(eg[:], vals[g][:], wm2[:], Alu.is_equal)
                    mg = tp.tile([P, F], u8, tag=f"m{g}")
                    nc.vector.tensor_tensor(mg[:], eg[:], keep[g][:], Alu.mult)
                    m.append(mg)

                o = io.tile([P, F, 3], f32, tag="o")
                nc.scalar.memzero(o[:])
                # priority: group 0 wins ties -> write it last
                for g in (2, 1, 0):
                    nc.vector.copy_predicated(
                        o[:], m[g][:].broadcast_to((P, F, 3)), x[:, :, 3 * g : 3 * g + 3]
                    )
                nc.sync.dma_start(ot[t], o[:])
    nc.compile()
    return nc


def _run(full_inputs: np.ndarray, trace: bool = False):
    global LAST_EXEC_NS, LAST_RESULTS
    from concourse.bass_utils import run_bass_kernel_spmd

    if "nc" not in _CACHE:
        _CACHE["nc"] = _build_nc()
    nc = _CACHE["nc"]

    shards = full_inputs.reshape(N_CORES, ROWS_PER_CORE, 9)
    in_maps = [{"inputs": np.ascontiguousarray(shards[i])} for i in range(N_CORES)]
    res = run_bass_kernel_spmd(nc, in_maps, list(range(N_CORES)), trace=trace)
    LAST_EXEC_NS = res.exec_time_ns
    LAST_RESULTS = res
    out = np.concatenate([res.results[i]["out"] for i in range(N_CORES)], axis=0)
    return out


def kernel(inputs: np.ndarray) -> np.ndarray:
    inputs = np.ascontiguousarray(np.asarray(inputs, dtype=np.float32))
    assert inputs.shape == (N_ROWS, 9), inputs.shape
    trace = bool(int(os.environ.get("BASS_KERNEL_TRACE", "0")))
    return _run(inputs, trace=trace)
